# revision 58
# baseline (speedup 1.0000x reference)
"""Trainium2 Bass kernel for nn_Estor_concat (scatter_memory).

Math (exact reformulation of the reference):
  The attention output for a span of tag t is the per-tag constant
  v_tag[t] = out_proj(V_proj(tag_emb[t])) (softmax over one logit == 1),
  so the FFN input reduces to counts[t, s] * v_tag[t] concatenated over
  tags, and the first FFN layer collapses to the [T, H] weight-only
  constant W_eff[t, j] = sum_h v_tag[t, h] * ff1_w[j, t*H + h], folded on
  the host (constant folding, like BN-folding).  Per batch b the device
  computes:
    counts[t, s] = #spans(tag t) covering s
                 = sum_n oht[n,t]*(s >= start_n) - oht[n,t]*(s >= end_n)
    h1 = relu(W_eff.T @ counts + ff1_b)          [H, S]
    h2 = ff2 @ h1 + ff2_b                        [H2, S]  (fp8 DoubleRow)
    raw = [lwg_we | lwg_h2].T @ [we; h2]         [NL+1, S] (+ sum row)
    out = (raw - c1*mu + c2*sd) / bcast(sd)      (LayerNorm folded into
                                                  the output projection)
  with lwg = (lin_w * ln_g).T, c1 = col-sums of lwg, c2 = lin_w@ln_b+lin_b.

Sharding: pure data-parallel over batch (8 cores, 1 batch each), no
collectives; all post-fold weights are small and replicated.
"""

import ml_dtypes
import numpy as np

import concourse.bacc as bacc
import concourse.bass as bass
import concourse.mybir as mybir
import concourse.tile as tile
from concourse.bass_utils import run_bass_kernel_spmd

T, B, S, H = 16, 8, 512, 768
H2 = 384
NEW_H = H + H2          # 1152
NL = 33                 # num labels
NCORES = 8
KC_H = H // 128         # 6 chunks of the hidden dim
KC_H2 = H2 // 128       # 3
P = 128
M_PR = NL + 1           # 34: label rows + ones (sum) row
FF2_SCALE = 64.0        # fp8 pre-scale keeping ff2 out of e4m3 subnormals
H1_DIV = 4.0            # h1r stored /4 so prh2_w*4 clears fp8 subnormals
H2_SCALE = FF2_SCALE / H1_DIV   # h2 psum arrives scaled by this
LWG_W = KC_H * M_PR             # lwg_we chunks
F8_W = H2 + M_PR                # ff2 | prh2 packed width

F32 = mybir.dt.float32
BF16 = mybir.dt.bfloat16
F16 = mybir.dt.float16
F8 = mybir.dt.float8e4
DR = mybir.MatmulPerfMode.DoubleRow
ALU = mybir.AluOpType
ACT = mybir.ActivationFunctionType


def build_kernel(nt: int):
    nc = bacc.Bacc(
        "TRN2",
        target_bir_lowering=False,
        debug=False,
        enable_asserts=True,
        num_devices=NCORES,
    )

    def inp(name, shape, dtype=F32):
        return nc.dram_tensor(name, list(shape), dtype, kind="ExternalInput").ap()

    # packed inputs (few DMAs; see host prep for layouts)
    pk32 = inp("pk32", (P, 2 * nt + KC_H + KC_H2))  # sps | spe | ff1b | ff2b
    oht = inp("oht", (P, nt * 2 * T), BF16)         # [+onehot | -onehot]
    lwg = inp("lwg", (P, LWG_W + 2 * NL + M_PR), BF16)  # lwg | c1n | c2 | cb
    weff = inp("weff", (T, KC_H, P), BF16)          # W_eff[t, kj*128+m] / 4
    ff2t = inp("ff2t", (P, KC_H, F8_W), F8)         # ff2.T*64 | prh2_w.T*4
    we_t = inp("we_t", (P, KC_H, S), BF16)          # word_embedding[b].T

    out = nc.dram_tensor("out", [NL, S], F32, kind="ExternalOutput").ap()

    with tile.TileContext(nc) as tc:
        with (
            tc.tile_pool(name="singles", bufs=1) as singles,
            tc.tile_pool(name="spans", bufs=5) as spans,
            tc.tile_pool(name="ps_acc", bufs=1, space="PSUM") as ps_acc,
            tc.tile_pool(name="ps_h1", bufs=2, space="PSUM") as ps_h1,
            tc.tile_pool(name="ps_h2", bufs=2, space="PSUM") as ps_h2,
        ):
            # ---- constants ----
            ones_col = singles.tile([P, 1], BF16)
            nc.vector.memset(ones_col, 1.0)
            ones_row = singles.tile([1, NL], F16)
            nc.vector.memset(ones_row, 1.0)
            ones512 = singles.tile([1, S], F16)
            nc.vector.memset(ones512, 1.0)
            eps_t = singles.tile([1, 1], F32)
            nc.vector.memset(eps_t, 0.0)
            scratch = singles.tile([1, 1], F32)
            warm_sb = singles.tile([P, S], BF16)
            nc.gpsimd.memset(warm_sb, 0.25)
            # iota generated on-device: cheaper than a DMA (no 900ns sem)
            iota = singles.tile([P, S], F16)
            nc.gpsimd.iota(iota, [[1, S]], base=0, channel_multiplier=0,
                           allow_small_or_imprecise_dtypes=True)

            # ---- DMAs: mask-path + we lead the HWDGE queue; the small
            # weight tensors ride the Pool SWDGE queue in parallel ----
            pk32_sb = singles.tile([P, 2 * nt + KC_H + KC_H2], F32)
            nc.sync.dma_start(out=pk32_sb, in_=pk32)
            oht_sb = singles.tile([P, nt * 2 * T], BF16)
            nc.sync.dma_start(out=oht_sb, in_=oht)
            we_sb = singles.tile([P, KC_H, S], BF16)
            nc.sync.dma_start(out=we_sb[:, 0:3, :], in_=we_t[:, 0:3, :])
            nc.sync.dma_start(out=we_sb[:, 3:6, :], in_=we_t[:, 3:6, :])
            weff_sb = singles.tile([T, KC_H, P], BF16)
            nc.gpsimd.dma_start(out=weff_sb, in_=weff)
            lwg_sb = singles.tile([P, LWG_W + 2 * NL + M_PR], BF16)
            nc.gpsimd.dma_start(out=lwg_sb, in_=lwg)
            ff2_sb = singles.tile([P, KC_H, F8_W], F8)
            nc.gpsimd.dma_start(out=ff2_sb, in_=ff2t)

            def sps_col(i):
                return pk32_sb[:, i:i + 1]

            def spe_col(i):
                return pk32_sb[:, nt + i:nt + i + 1]

            def ff1b_col(kj):
                return pk32_sb[:, 2 * nt + kj:2 * nt + kj + 1]

            def ff2b_col(mc):
                return pk32_sb[:, 2 * nt + KC_H + mc:2 * nt + KC_H + mc + 1]

            def oht_pos(i):
                return oht_sb[:, i * 2 * T:i * 2 * T + T]

            def oht_neg(i):
                return oht_sb[:, i * 2 * T + T:(i + 1) * 2 * T]

            def lwg_c(fc):
                return lwg_sb[:, fc * M_PR:(fc + 1) * M_PR]

            c1n_row = lwg_sb[0:1, LWG_W:LWG_W + NL]
            c2_row = lwg_sb[0:1, LWG_W + NL:LWG_W + 2 * NL]
            cb_row = lwg_sb[0:1, LWG_W + 2 * NL:LWG_W + 2 * NL + M_PR]

            # act-table warm-up (Square/Relu/Identity share one set; Sqrt
            # is avoided via DVE pow so only one table load happens)
            nc.scalar.activation(out=scratch, in_=eps_t, func=ACT.Square)

            # ---- PE p-state warm-up: reach 2.4 GHz before real work ----
            warm_ps = ps_acc.tile([1, S], F32, tag="sdb")
            for _ in range(3):
                nc.tensor.matmul(warm_ps, ones_col, warm_sb,
                                 start=True, stop=True)

            # ---- counts: (s>=start) - (s>=end) scatter on PE ----
            counts_ps = ps_acc.tile([T, S], F32, tag="counts")
            for i in range(nt):
                ge_s = spans.tile([P, S], BF16, tag="ge_s")
                nc.vector.tensor_scalar(
                    out=ge_s, in0=iota, scalar1=sps_col(i), scalar2=None,
                    op0=ALU.is_ge,
                )
                ge_e = spans.tile([P, S], BF16, tag="ge_e")
                nc.vector.tensor_scalar(
                    out=ge_e, in0=iota, scalar1=spe_col(i), scalar2=None,
                    op0=ALU.is_ge,
                )
                nc.tensor.matmul(counts_ps, oht_pos(i), ge_s,
                                 start=(i == 0), stop=False)
                nc.tensor.matmul(counts_ps, oht_neg(i), ge_e,
                                 start=False, stop=(i == nt - 1))
            counts_sb = singles.tile([T, S], BF16)
            nc.vector.tensor_copy(out=counts_sb, in_=counts_ps)

            # ---- raw accumulation: constant cb opens the group ----
            pr_ps = ps_acc.tile([M_PR, S], F32, tag="pr")
            nc.tensor.matmul(pr_ps, cb_row, ones512, start=True, stop=False)
            for fc in range(3):
                nc.tensor.matmul(pr_ps, lwg_c(fc), we_sb[:, fc, :],
                                 start=False, stop=False)

            # ---- h1 = relu(W_eff.T @ counts + ff1_b) -> fp8 [H, S] ----
            h1r_sb = singles.tile([P, KC_H, S], F8)
            relu_eng = ["pool", "act", "pool", "act", "pool", "vec"]
            h1_ps_l = []
            for kj in range(KC_H):
                if kj == 2:
                    # extra buffers via the freed counts and warm banks
                    ps = ps_acc.tile([P, S], F32, tag="counts")
                elif kj == 3:
                    ps = ps_acc.tile([P, S], F32, tag="sdb")
                else:
                    ps = ps_h1.tile([P, S], F32, tag="h1")
                h1_ps_l.append(ps)
                nc.tensor.matmul(ps, weff_sb[:, kj, :], counts_sb,
                                 start=True, stop=True)
            for kj in range(KC_H):
                ps = h1_ps_l[kj]
                if relu_eng[kj] == "vec":
                    nc.vector.tensor_scalar(
                        out=h1r_sb[:, kj, :], in0=ps, scalar1=ff1b_col(kj),
                        scalar2=0.0, op0=ALU.add, op1=ALU.max)
                elif relu_eng[kj] == "act":
                    nc.scalar.activation(
                        out=h1r_sb[:, kj, :], in_=ps, func=ACT.Relu,
                        bias=ff1b_col(kj))
                else:
                    nc.gpsimd.tensor_scalar(
                        out=h1r_sb[:, kj, :], in0=ps, scalar1=ff1b_col(kj),
                        scalar2=0.0, op0=ALU.add, op1=ALU.max)

            for fc in range(3, KC_H):
                nc.tensor.matmul(pr_ps, lwg_c(fc), we_sb[:, fc, :],
                                 start=False, stop=False)

            # ---- we squares (fp8 pairs enable DoubleRow variance mms) ----
            sqp1 = singles.tile([P, 2, S], F8)
            sqp2 = singles.tile([P, 2, S], F8)
            sqw4 = singles.tile([P, S], BF16)
            sqw5 = singles.tile([P, S], BF16)
            nc.gpsimd.tensor_tensor(
                out=sqp1[:, 0, :], in0=we_sb[:, 0, :],
                in1=we_sb[:, 0, :], op=ALU.mult)
            nc.gpsimd.tensor_tensor(
                out=sqp1[:, 1, :], in0=we_sb[:, 1, :],
                in1=we_sb[:, 1, :], op=ALU.mult)
            nc.gpsimd.tensor_tensor(
                out=sqp2[:, 0, :], in0=we_sb[:, 2, :],
                in1=we_sb[:, 2, :], op=ALU.mult)
            nc.gpsimd.tensor_tensor(
                out=sqp2[:, 1, :], in0=we_sb[:, 3, :],
                in1=we_sb[:, 3, :], op=ALU.mult)
            nc.scalar.activation(
                out=sqw4, in_=we_sb[:, 4, :], func=ACT.Square)
            nc.scalar.activation(
                out=sqw5, in_=we_sb[:, 5, :], func=ACT.Square)

            # ---- sum of squares, we part (early, fills PE gaps) ----
            ones_pair = singles.tile([P, 2, 1], F8)
            nc.vector.memset(ones_pair, 1.0)
            ss_ps = ps_acc.tile([1, S], F32, tag="ss")
            nc.tensor.matmul(ss_ps, ones_pair, sqp1, start=True, stop=False,
                             perf_mode=DR)
            nc.tensor.matmul(ss_ps, ones_pair, sqp2, start=False, stop=False,
                             perf_mode=DR)
            nc.tensor.matmul(ss_ps, ones_col, sqw4, start=False, stop=False)
            nc.tensor.matmul(ss_ps, ones_col, sqw5, start=False, stop=False)

            # ---- h2 = ff2 @ relu_h1 (fp8 DoubleRow) ----
            h2sqp = singles.tile([P, 4, S], F8)
            nc.gpsimd.memset(h2sqp[:, 3, :], 0.0)
            h2_ps_l = []
            for mc in range(KC_H2):
                # third buffer for mc2 via the freed counts bank
                if mc == 2:
                    ps = ps_acc.tile([P, S], F32, tag="counts")
                else:
                    ps = ps_h2.tile([P, S], F32, tag="h2")
                h2_ps_l.append(ps)
            # kt-outer: only the kt2 round (and the last prh2 pair) waits
            # for the final relu; everything else overlaps the relu service
            for kt in range(KC_H // 2):
                for mc in range(KC_H2):
                    nc.tensor.matmul(
                        h2_ps_l[mc],
                        ff2_sb[:, 2 * kt:2 * kt + 2, mc * P:(mc + 1) * P],
                        h1r_sb[:, 2 * kt:2 * kt + 2, :],
                        start=(kt == 0), stop=(kt == KC_H // 2 - 1),
                        perf_mode=DR,
                    )
                # pr_h2 via folded fp8 weights: prh2_w.T @ relu(h1)
                nc.tensor.matmul(
                    pr_ps, ff2_sb[:, 2 * kt:2 * kt + 2, H2:H2 + M_PR],
                    h1r_sb[:, 2 * kt:2 * kt + 2, :],
                    start=False, stop=(kt == KC_H // 2 - 1), perf_mode=DR)

            # biased squares straight from psum (h2 values only feed the
            # variance; the label projection reads h1r via folded weights)
            nc.scalar.activation(
                out=h2sqp[:, 0, :], in_=h2_ps_l[0], func=ACT.Square,
                bias=ff2b_col(0), scale=1.0 / H2_SCALE)
            h2m1 = singles.tile([P, S], BF16)
            nc.gpsimd.tensor_scalar(
                out=h2m1, in0=h2_ps_l[1], scalar1=1.0 / H2_SCALE,
                scalar2=ff2b_col(1), op0=ALU.mult, op1=ALU.add)
            nc.gpsimd.tensor_tensor(
                out=h2sqp[:, 1, :], in0=h2m1, in1=h2m1, op=ALU.mult)
            nc.scalar.activation(
                out=h2sqp[:, 2, :], in_=h2_ps_l[2], func=ACT.Square,
                bias=ff2b_col(2), scale=1.0 / H2_SCALE)

            # ---- sum of squares, h2 part (slot 3 is zero padding) ----
            nc.tensor.matmul(ss_ps, ones_pair, h2sqp[:, 0:2, :],
                             start=False, stop=False, perf_mode=DR)
            nc.tensor.matmul(ss_ps, ones_pair, h2sqp[:, 2:4, :],
                             start=False, stop=True, perf_mode=DR)

            # ---- LayerNorm stats: DVE back-to-back, no engine hops ----
            sumrow = singles.tile([1, S], F16)
            nc.vector.tensor_copy(out=sumrow, in_=pr_ps[NL:NL + 1, :])
            # -c1*mu rides the pr psum accumulation (group re-opened)
            nc.tensor.matmul(pr_ps[0:NL, :], c1n_row, sumrow,
                             start=False, stop=False, skip_group_check=True)
            mu2 = singles.tile([1, S], F16)
            nc.vector.tensor_scalar(
                out=mu2, in0=sumrow, scalar1=1.0 / NEW_H, scalar2=2.0,
                op0=ALU.mult, op1=ALU.pow)
            var_sb = singles.tile([1, S], F32)
            HV = S // 2
            nc.vector.scalar_tensor_tensor(
                out=var_sb[:, 0:HV], in0=ss_ps[:, 0:HV], scalar=1.0 / NEW_H,
                in1=mu2[:, 0:HV], op0=ALU.mult, op1=ALU.subtract)
            nc.gpsimd.scalar_tensor_tensor(
                out=var_sb[:, HV:S], in0=ss_ps[:, HV:S], scalar=1.0 / NEW_H,
                in1=mu2[:, HV:S], op0=ALU.mult, op1=ALU.subtract)
            sd = singles.tile([1, S], F16)
            nc.vector.tensor_scalar(
                out=sd[:, 0:HV], in0=var_sb[:, 0:HV], scalar1=0.5,
                scalar2=None, op0=ALU.pow)
            nc.gpsimd.tensor_scalar(
                out=sd[:, HV:S], in0=var_sb[:, HV:S], scalar1=0.5,
                scalar2=None, op0=ALU.pow)
            # +c2*sd closes the pr group
            nc.tensor.matmul(pr_ps[0:NL, :], c2_row, sd,
                             start=False, stop=True, skip_group_check=True)
            sdb_ps = ps_h2.tile([NL, S], F32, tag="h2")
            nc.tensor.matmul(sdb_ps, ones_row, sd, start=True, stop=True)

            # ---- final: (raw - c1*mu + c2*sd) / sd ----
            # asymmetric pieces on two DMA queues: the later piece is
            # smaller, so its divide (and the final semaphore) lands early
            f_sb = singles.tile([NL, S], F32)
            HS = 352
            nc.vector.tensor_tensor(
                out=f_sb[:, 0:HS], in0=pr_ps[0:NL, 0:HS],
                in1=sdb_ps[:, 0:HS], op=ALU.divide)
            nc.sync.dma_start(out=out[:, 0:HS], in_=f_sb[:, 0:HS])
            nc.gpsimd.tensor_tensor(
                out=f_sb[:, HS:S], in0=pr_ps[0:NL, HS:S],
                in1=sdb_ps[:, HS:S], op=ALU.divide)
            nc.scalar.dma_start(out=out[:, HS:S], in_=f_sb[:, HS:S])

    nc.compile()
    return nc


def _chunked(a, kc):
    """[kc*128, N...] -> [128, kc, N...] (partition-major chunk layout)."""
    return np.ascontiguousarray(
        a.reshape(kc, P, *a.shape[1:]).transpose(1, 0, *range(2, a.ndim + 1))
    )


_CACHE = {}


def kernel(**inputs) -> np.ndarray:
    bfl = ml_dtypes.bfloat16
    f8 = ml_dtypes.float8_e4m3
    we = np.asarray(inputs["word_embedding"], np.float32)
    te = np.asarray(inputs["tag_embedding"], np.float32)
    ipw = np.asarray(inputs["in_proj_w"], np.float32)
    ipb = np.asarray(inputs["in_proj_b"], np.float32)
    opw = np.asarray(inputs["out_proj_w"], np.float32)
    ob_ = np.asarray(inputs["out_proj_b"], np.float32)
    f1w = np.asarray(inputs["ff1_w"], np.float32)
    f1b = np.asarray(inputs["ff1_b"], np.float32)
    f2w = np.asarray(inputs["ff2_w"], np.float32)
    f2b = np.asarray(inputs["ff2_b"], np.float32)
    lg = np.asarray(inputs["ln_g"], np.float32)
    lb = np.asarray(inputs["ln_b"], np.float32)
    lw = np.asarray(inputs["lin_w"], np.float32)
    lbias = np.asarray(inputs["lin_b"], np.float32)
    sb = np.asarray(inputs["span_batch"]).astype(np.int64)
    st = np.asarray(inputs["span_tag"]).astype(np.int64)
    ss = np.asarray(inputs["span_start"]).astype(np.int64)
    se = np.asarray(inputs["span_end"]).astype(np.int64)

    # ---- weight-only constant folding (host) --------------------------
    v_tag = (te @ ipw[2 * H:].T + ipb[2 * H:]) @ opw.T + ob_   # [T, H]
    weff = np.stack(
        [f1w[:, t * H:(t + 1) * H] @ v_tag[t] for t in range(T)])
    weff_c = np.ascontiguousarray(
        (weff / H1_DIV).reshape(T, KC_H, P).astype(bfl))

    lwgT = (lw * lg).T                                   # [NEW_H, NL]
    lwg_np = np.zeros((P, KC_H, M_PR), bfl)              # we-part lhsT
    lwg_np[:, :, :NL] = _chunked(lwgT[:H].astype(bfl), KC_H)
    lwg_np[:, :, NL] = 1.0
    c1n_np = (-lwgT.sum(0) / NEW_H).astype(bfl)          # [NL]
    c2_np = (lw @ lb + lbias).astype(bfl)
    # fold lwg_h2.T @ ff2: the label projection reads relu(h1) directly
    lwg_h2 = np.concatenate(
        [lwgT[H:], np.ones((H2, 1), np.float32)], axis=1)    # [H2, 34]
    prh2_full = lwg_h2.T @ f2w                               # [34, H]
    cb_np = (lwg_h2.T @ f2b).astype(bfl)                     # [34]

    ff2t_np = np.zeros((P, KC_H, F8_W), f8)
    ff2t_np[:, :, :H2] = _chunked((f2w.T * FF2_SCALE).astype(f8), KC_H)
    ff2t_np[:, :, H2:] = _chunked(
        np.ascontiguousarray(prh2_full.T * H1_DIV).astype(f8), KC_H)
    ff1b_np = np.ascontiguousarray(f1b.reshape(KC_H, P).T) / H1_DIV
    ff2b_np = np.ascontiguousarray(f2b.reshape(KC_H2, P).T)

    counts_per_b = np.bincount(sb, minlength=B)
    nt = max(1, int(np.ceil(counts_per_b.max() / P)))
    n_pad = nt * P

    in_maps = []
    for c in range(NCORES):
        idx = np.where(sb == c)[0]
        n = len(idx)
        pk32 = np.zeros((P, 2 * nt + KC_H + KC_H2), np.float32)
        sps_np = np.zeros(n_pad, np.float32)
        spe_np = np.zeros(n_pad, np.float32)
        oht_np = np.zeros((n_pad, 2 * T), bfl)
        sps_np[:n] = ss[idx]
        spe_np[:n] = se[idx]
        oht_np[np.arange(n), st[idx]] = 1.0
        oht_np[np.arange(n), T + st[idx]] = -1.0
        pk32[:, 0:nt] = sps_np.reshape(nt, P).T
        pk32[:, nt:2 * nt] = spe_np.reshape(nt, P).T
        pk32[:, 2 * nt:2 * nt + KC_H] = ff1b_np
        pk32[:, 2 * nt + KC_H:] = ff2b_np
        oht_pk = np.ascontiguousarray(
            oht_np.reshape(nt, P, 2 * T).transpose(1, 0, 2)
            .reshape(P, nt * 2 * T))
        lwg_pk = np.zeros((P, LWG_W + 2 * NL + M_PR), bfl)
        lwg_pk[:, :LWG_W] = lwg_np.reshape(P, LWG_W)
        lwg_pk[0, LWG_W:LWG_W + NL] = c1n_np
        lwg_pk[0, LWG_W + NL:LWG_W + 2 * NL] = c2_np
        lwg_pk[0, LWG_W + 2 * NL:] = cb_np
        in_maps.append(dict(
            pk32=pk32, oht=oht_pk, lwg=lwg_pk, weff=weff_c, ff2t=ff2t_np,
            we_t=_chunked(np.ascontiguousarray(we[c].T).astype(bfl), KC_H),
        ))

    if nt not in _CACHE:
        _CACHE[nt] = build_kernel(nt)
    nc = _CACHE[nt]

    res = run_bass_kernel_spmd(nc, in_maps, list(range(NCORES)))
    out = np.stack([res.results[c]["out"].T for c in range(NCORES)])
    return out.astype(np.float32)


if __name__ == "__main__":
    import reference
    inp = {k: np.asarray(v) for k, v in reference.setup_inputs().items()}
    got = kernel(**inp)
    print("kernel output:", got.shape, got.dtype)
